# revision 43
# baseline (speedup 1.0000x reference)
"""Multi-head self-attention (RoPE + causal) Trainium2 Bass kernel.

Problem: b=2, s=2048, d_model=1024, 16 heads x 64 dims, causal, RoPE.
Sharding over 8 NeuronCores: core c -> (batch bi = c//4, head group g = c%4
of 4 heads). Each core computes its 4 heads' attention from x[bi] and
produces a partial output projection (Wo column-block); the host sums the
4 partials per batch element.

v3: single interleaved pipeline tuned to keep the PE HAM-warm (2.4 GHz):
per s-quarter c: QK proj -> rope -> attention (qi=c), with V-projection and
out-projection tiles of the previous quarter emitted as PE fillers inside
the attention ladder (scores run one block ahead of AV so the PE never
waits on the ACT exp). Inputs arrive via consolidated multi-dim DMAs.
Causal diag masking is a bf16 triangular-mask multiply on DVE. Softmax
division: denominator row staged to partition 0 (custom-DVE reciprocal
mishandles nonzero partition bases), reciprocal, gpsimd partition
broadcast, multiply.

Per-core device layout (all matmul operands bf16, fp32 PSUM accumulate):
  x_all  [128, 4*8*512]   xT quarters: [quarter][kc][512 cols]
  w_qk   [128, 8*512]     per kc: [Q pair0 | Q pair1 | K pair0 | K pair1]
                          rows permuted per pair: [h even, h odd, h' even,
                          h' odd] (32 rows each) so RoPE is a half-swap +
                          elementwise mul/add
  QT/KT  [128, 2048]x2    projected+roped, tile t holds heads 2t, 2t+1
  scores ST[k,q] via PE (contraction = head dims, row-group packed pairs)
  softmax: exp on ACT (scale=1/8 folded in), causal mask = multiplicative
           bf16 upper-tri tile, denominator = ones column appended to V
           (row 64 of the AV PSUM accumulator)
  out    [2048, 1024] bf16 partial = O @ Wo_block
"""

import os
import sys
from contextlib import ExitStack

import numpy as np

for _p in ("/root/.axon_site", "/root/.axon_site/_ro/trn_rl_repo", "/opt/trn_rl_repo"):
    if os.path.isdir(_p) and _p not in sys.path:
        sys.path.append(_p)

import ml_dtypes  # noqa: E402
import concourse.bass as bass  # noqa: E402
import concourse.tile as tile  # noqa: E402
import concourse.mybir as mybir  # noqa: E402
from concourse import bacc  # noqa: E402
from concourse.bass import ts  # noqa: E402
from concourse.bass_utils import run_bass_kernel_spmd  # noqa: E402

BF16 = mybir.dt.bfloat16
F32 = mybir.dt.float32
NPBF16 = ml_dtypes.bfloat16

S = 2048
D = 1024
DK = 64
THETA = 10000.0

_CACHE = {}


def _build_nc(debug_out=False):
    nc = bacc.Bacc("TRN2", target_bir_lowering=False, debug=False, num_devices=8)
    xT = nc.dram_tensor("xT", [D, S], BF16, kind="ExternalInput").ap()
    wqkv = nc.dram_tensor("wqkv", [D, 768], BF16, kind="ExternalInput").ap()
    woT = nc.dram_tensor("woT", [256, D], BF16, kind="ExternalInput").ap()
    ropec = nc.dram_tensor("ropec", [128, 2048], BF16, kind="ExternalInput").ap()
    ropes = nc.dram_tensor("ropes", [128, 2048], BF16, kind="ExternalInput").ap()
    trimask = nc.dram_tensor("trimask", [128, 128], BF16, kind="ExternalInput").ap()
    yp = nc.dram_tensor("yp", [S, D], BF16, kind="ExternalOutput").ap()
    dbg = {}
    if debug_out:
        for nm in ("qf0", "kf0", "ot0", "ot1"):
            dbg[nm] = nc.dram_tensor(nm, [128, S], BF16, kind="ExternalOutput").ap()

    Exp = mybir.ActivationFunctionType.Exp

    with ExitStack() as ctx:
        tc = ctx.enter_context(tile.TileContext(nc))
        const = ctx.enter_context(tc.tile_pool(name="const", bufs=1))
        sb = ctx.enter_context(tc.tile_pool(name="sb", bufs=2))
        expp = ctx.enter_context(tc.tile_pool(name="expp", bufs=4))
        outp = ctx.enter_context(tc.tile_pool(name="outp", bufs=3))
        rp = ctx.enter_context(tc.tile_pool(name="rp", bufs=4))
        psA = ctx.enter_context(tc.tile_pool(name="psA", bufs=2, space="PSUM"))
        psB = ctx.enter_context(tc.tile_pool(name="psB", bufs=4, space="PSUM"))

        # ---- persistent SBUF ----
        x_all = const.tile([128, 4 * 8 * 512], BF16, tag="x_all")
        w_qk = const.tile([128, 8 * 512], BF16, tag="w_qk")
        w_v = const.tile([128, 8 * 256], BF16, tag="w_v")
        wo_all = const.tile([128, 2 * 1024], BF16, tag="wo_all")
        ropec_sb = const.tile([128, 2048], BF16, tag="ropec")
        ropes_sb = const.tile([128, 2048], BF16, tag="ropes")
        trimask_sb = const.tile([128, 128], BF16, tag="trimask")
        # V slots are 128 wide: [ones | 63 pad | 64 dims] so the AV PSUM
        # accumulator has the denominator on partition 0 (custom-DVE recip
        # requires base 0) and the dims at base 64 (aligned 64-partition read)
        v_sb = const.tile([128, 16 * 512], BF16, tag="v")
        qf = [const.tile([128, S], BF16, tag=f"qf{t}", name=f"qf{t}") for t in range(2)]
        kf = [const.tile([128, S], BF16, tag=f"kf{t}", name=f"kf{t}") for t in range(2)]
        ot = [const.tile([128, S], BF16, tag=f"ot{t}", name=f"ot{t}") for t in range(2)]

        def xq(kc, c):  # x columns for quarter c, contraction tile kc
            o = (c * 8 + kc) * 512
            return x_all[:, o : o + 512]

        def xst(kc, st):  # x columns for s-tile st (128 wide)
            c, si = divmod(st, 4)
            o = (c * 8 + kc) * 512 + si * 128
            return x_all[:, o : o + 128]

        def wqk(kc, t, qk):  # Q (qk=0) / K (qk=1) weight tile for pair t
            o = kc * 512 + qk * 256 + t * 128
            return w_qk[:, o : o + 128]

        def wv(kc):
            return w_v[:, kc * 256 : (kc + 1) * 256]

        def wo(cc, nh):
            o = cc * 1024 + nh * 512
            return wo_all[:, o : o + 512]

        # ---- consolidated input DMAs, split across the SP and ACT HW-DGE
        # queues so transfers run in parallel; quarter 0 lands first ----
        xT_r = xT[:].rearrange("(k p) s -> p k s", k=8)
        wq_r = wqkv[:].rearrange("(k p) c -> p k c", k=8)
        wqk_4d = w_qk[:].rearrange("p (k q c) -> p k q c", k=8, q=4)
        wqs_4d = wq_r[:, :, 0:512].rearrange("p k (q c) -> p k q c", q=4)
        # pair-0 weight slices first (Q t / K t are 128-col slices t, 2+t)
        for q, eng in ((0, nc.sync), (2, nc.sync), (1, nc.scalar), (3, nc.scalar)):
            eng.dma_start(wqk_4d[:, :, q : q + 1, :], wqs_4d[:, :, q : q + 1, :])
        x0 = x_all[:, 0:4096].rearrange("p (k s) -> p k s", k=8)
        nc.sync.dma_start(x0[:, 0:4, :], xT_r[:, 0:4, 0:512])
        nc.scalar.dma_start(x0[:, 4:8, :], xT_r[:, 4:8, 0:512])
        nc.sync.dma_start(ropec_sb[:], ropec[:])
        nc.scalar.dma_start(ropes_sb[:], ropes[:])
        nc.sync.dma_start(trimask_sb[:], trimask[:])
        nc.scalar.dma_start(
            w_v[:].rearrange("p (k c) -> p k c", k=8), wq_r[:, :, 512:768]
        )
        # bulk loads go via the gpsimd software-DGE queue so the SP/ACT HW
        # queues stay clear for the latency-critical rope swap DMAs
        for c in range(1, 4):
            nc.gpsimd.dma_start(
                x_all[:, c * 4096 : (c + 1) * 4096].rearrange("p (k s) -> p k s", k=8),
                xT_r[:, :, ts(c, 512)],
            )
        nc.gpsimd.dma_start(
            wo_all[:].rearrange("p (k c) -> p k c", k=2),
            woT[:].rearrange("(k p) c -> p k c", k=2),
        )

        # v_sb slot headers: ones at col 0, zero pad at cols 1:32
        v_4d = v_sb[:].rearrange("p (b h x) -> p b h x", b=16, h=4)
        nc.gpsimd.memset(v_4d[:, :, :, 0:1], 1.0)
        nc.gpsimd.memset(v_4d[:, :, :, 1:64], 0.0)

        # deferred divide state from the previous (c, t) attention pass
        pend_div = []  # list of (qi, t, [oa0, oa1], [r0, r1])

        def emit_divides(upto):
            """Emit pbcast+mul for pending divides (recip already emitted)."""
            while len(pend_div) > upto:
                qi, t, oas, rs = pend_div.pop(0)
                for hh in range(2):
                    rb = rp.tile([64, 512], F32, tag="rb", name="rb")
                    nc.gpsimd.partition_broadcast(rb[:], rs[hh][0:1, :])
                    nc.vector.tensor_mul(
                        ot[t][64 * hh : 64 * hh + 64, ts(qi, 512)],
                        oas[hh][64:128, :],
                        rb[:],
                    )

        def emit_vproj(st):
            vp = psA.tile([128, 1024], F32, tag="psa", name="vp")
            for kc in range(8):
                nc.tensor.matmul(
                    vp[:, 0:256],
                    lhsT=xst(kc, st),
                    rhs=wv(kc),
                    start=(kc == 0),
                    stop=(kc == 7),
                )
            dst = v_sb[:, st * 512 : (st + 1) * 512].rearrange(
                "p (h x) -> p h x", h=4
            )[:, :, 64:128]
            vsrc = vp[:, 0:256].rearrange("p (h x) -> p h x", h=4)
            nc.vector.tensor_copy(dst, vsrc)

        def emit_outproj(st):
            pp = psA.tile([128, 1024], F32, tag="psa", name="pp")
            for nh in range(2):
                for cc in range(2):
                    nc.tensor.matmul(
                        pp[:, ts(nh, 512)],
                        lhsT=ot[cc][:, ts(st, 128)],
                        rhs=wo(cc, nh),
                        start=(cc == 0),
                        stop=(cc == 1),
                    )
            ob = outp.tile([128, 1024], BF16, tag="ob", name="ob")
            nc.scalar.copy(ob[:, 0:512], pp[:, 0:512])
            nc.vector.tensor_copy(ob[:, 512:1024], pp[:, 512:1024])
            nc.sync.dma_start(yp[ts(st, 128), :], ob[:])

        def qkproj_fillers(c, t):
            """Q/K projection + RoPE for quarter c, head pair t, as two PE
            filler chunks (Q half, then K half + the rope chain). The rope
            elementwise chain runs on DVE; the half-swap runs as SBUF->SBUF
            DMAs to keep it off the busy compute engines."""
            st8 = {}

            def f_q():
                sp = psA.tile([128, 1024], F32, tag="psa", name="sp")
                st8["sp"] = sp
                for kc in range(8):
                    nc.tensor.matmul(
                        sp[:, 0:512],
                        lhsT=wqk(kc, t, 0),
                        rhs=xq(kc, c),
                        start=(kc == 0),
                        stop=(kc == 7),
                    )

            def f_k():
                sp = st8["sp"]
                for kc in range(8):
                    nc.tensor.matmul(
                        sp[:, 512:1024],
                        lhsT=wqk(kc, t, 1),
                        rhs=xq(kc, c),
                        start=(kc == 0),
                        stop=(kc == 7),
                    )
                qb = sb.tile([128, 1024], BF16, tag="qb", name="qb")
                nc.vector.tensor_copy(qb[:], sp[:])
                wb = sb.tile([128, 1024], BF16, tag="wb", name="wb")
                nc.sync.dma_start(wb[0:32, :], qb[32:64, :])
                nc.sync.dma_start(wb[32:64, :], qb[0:32, :])
                nc.sync.dma_start(wb[64:96, :], qb[96:128, :])
                nc.sync.dma_start(wb[96:128, :], qb[64:96, :])
                t1 = sb.tile([128, 1024], BF16, tag="t1", name="t1")
                nc.vector.tensor_mul(t1[:, 0:512], qb[:, 0:512], ropec_sb[:, ts(c, 512)])
                nc.vector.tensor_mul(t1[:, 512:1024], qb[:, 512:1024], ropec_sb[:, ts(c, 512)])
                t2 = sb.tile([128, 1024], BF16, tag="t2", name="t2")
                nc.vector.tensor_mul(t2[:, 0:512], wb[:, 0:512], ropes_sb[:, ts(c, 512)])
                nc.vector.tensor_mul(t2[:, 512:1024], wb[:, 512:1024], ropes_sb[:, ts(c, 512)])
                nc.vector.tensor_add(qf[t][:, ts(c, 512)], t1[:, 0:512], t2[:, 0:512])
                nc.vector.tensor_add(kf[t][:, ts(c, 512)], t1[:, 512:1024], t2[:, 512:1024])

            return [f_q, f_k]

        # PE filler work queue, drained one item per attention block; the
        # next quarter's projections run as fillers inside this quarter's
        # ladder so the PE stays dense while ACT paces the exps. Leftovers
        # drain before the next ladder (whose scores depend on them).
        fillers = []

        def emit_filler():
            if fillers:
                fillers.pop(0)()

        # quarter-0 prologue
        for f in qkproj_fillers(0, 0) + qkproj_fillers(0, 1):
            f()
        for st in range(4):
            emit_vproj(st)

        for c in range(4):
            while fillers:
                emit_filler()
            if c < 3:
                fillers.extend(qkproj_fillers(c + 1, 0))
                fillers.extend(qkproj_fillers(c + 1, 1))
                fillers.extend(
                    [lambda st=st: emit_vproj(st) for st in range(4 * c + 4, 4 * c + 8)]
                )
            if c > 0:
                fillers.extend(
                    [lambda st=st: emit_outproj(st) for st in range(4 * (c - 1), 4 * c)]
                )

            # finish divides for the previous quarter
            emit_divides(0)

            # ---- attention for q-quarter qi=c, both head pairs merged into
            # one ladder (independent pairs interleave, so the PE never waits
            # on an ACT exp and there is no per-pair boundary bubble) ----
            qi = c
            nblk = 4 * qi + 4
            oab = [
                [psB.tile([128, 512], F32, tag="psb", name=f"oa{t}{_}") for _ in range(2)]
                for t in range(2)
            ]
            essb = [[None] * nblk, [None] * nblk]

            def emit_scores(t, j):
                dd = j - 4 * qi
                nn = 512 if dd < 0 else 512 - 128 * dd
                c0 = 512 - nn
                sp = psA.tile([128, 1024], F32, tag="psa", name="sc")
                for hh in range(2):
                    r0 = 64 * hh
                    nc.tensor.matmul(
                        sp[:, hh * 512 : hh * 512 + nn],
                        lhsT=kf[t][r0 : r0 + 64, ts(j, 128)],
                        rhs=qf[t][r0 : r0 + 64, qi * 512 + c0 : (qi + 1) * 512],
                        start=True,
                        stop=True,
                    )
                es = expp.tile([128, 1024], BF16, tag="es", name="es")
                essb[t][j] = es
                sp_v = sp[:].rearrange("p (b x) -> p b x", b=2)[:, :, 0:nn]
                es_v = es[:].rearrange("p (b x) -> p b x", b=2)[:, :, 0:nn]
                nc.scalar.activation(es_v, sp_v, Exp, scale=0.125)
                if dd >= 0:
                    for hh in range(2):
                        nc.vector.tensor_mul(
                            es[:, hh * 512 : hh * 512 + 128],
                            es[:, hh * 512 : hh * 512 + 128],
                            trimask_sb[:],
                        )

            def emit_av(t, j):
                dd = j - 4 * qi
                nn = 512 if dd < 0 else 512 - 128 * dd
                c0 = 512 - nn
                es = essb[t][j]
                for hh in range(2):
                    h = 2 * t + hh
                    off = j * 512 + h * 128
                    nc.tensor.matmul(
                        oab[t][hh][:, c0:512],
                        lhsT=v_sb[:, off : off + 128],
                        rhs=es[:, hh * 512 : hh * 512 + nn],
                        start=(j == 0),
                        stop=(j == nblk - 1),
                    )

            def emit_recips(t):
                # denominator reciprocals now (straight off the accumulator's
                # partition-0 row); pbcast+mul deferred so the PE can roll
                # into the next phase without waiting
                rs = []
                for hh in range(2):
                    r = rp.tile([1, 512], F32, tag="r", name="r")
                    nc.vector.reciprocal_approx_fast(r[:], oab[t][hh][0:1, 0:512])
                    rs.append(r)
                pend_div.append((qi, t, oab[t], rs))

            # split ladders per head pair, scores one block ahead of AV, with
            # a one-block lookahead across the pair boundary so the PE rolls
            # straight from pair 0's last AV into pair 1's first AV
            emit_scores(0, 0)
            for j in range(1, nblk):
                emit_scores(0, j)
                emit_filler()
                emit_av(0, j - 1)
            emit_scores(1, 0)
            emit_av(0, nblk - 1)
            emit_recips(0)
            for j in range(1, nblk):
                emit_scores(1, j)
                emit_filler()
                emit_av(1, j - 1)
            emit_av(1, nblk - 1)
            emit_recips(1)

        emit_divides(0)
        while fillers:
            emit_filler()
        for st in range(12, 16):
            emit_outproj(st)

        if debug_out:
            nc.sync.dma_start(dbg["qf0"][:], qf[0][:])
            nc.sync.dma_start(dbg["kf0"][:], kf[0][:])
            nc.sync.dma_start(dbg["ot0"][:], ot[0][:])
            nc.sync.dma_start(dbg["ot1"][:], ot[1][:])

    nc.compile()
    return nc


def _host_inputs(x, token_positions, Wq, Wk, Wv, Wo):
    x = np.asarray(x, dtype=np.float32)
    Wq = np.asarray(Wq, dtype=np.float32)
    Wk = np.asarray(Wk, dtype=np.float32)
    Wv = np.asarray(Wv, dtype=np.float32)
    Wo = np.asarray(Wo, dtype=np.float32)
    pos = np.asarray(token_positions).astype(np.float32)

    # rope tables, rows = [even(32) odd(32) even(32) odd(32)] freq index p%32
    f = np.arange(32, dtype=np.float32)
    inv = 1.0 / (THETA ** (2.0 * f / DK))
    ang = pos[:, None] * inv[None, :]  # [S, 32]
    cosT = np.cos(ang).T.astype(np.float32)  # [32, S]
    sinT = np.sin(ang).T.astype(np.float32)
    crow = np.tile(cosT, (4, 1))
    srow = np.concatenate([-sinT, sinT, -sinT, sinT], axis=0)

    ropec = np.ascontiguousarray(crow).astype(NPBF16)
    ropes = np.ascontiguousarray(srow).astype(NPBF16)
    trimask = np.triu(np.ones((128, 128), dtype=np.float32)).astype(NPBF16)

    ev = np.arange(0, DK, 2)
    od = np.arange(1, DK, 2)
    in_maps = []
    for core in range(8):
        bi, g = core // 4, core % 4
        xTb = np.ascontiguousarray(x[bi].T).astype(NPBF16)
        qk_idx = []
        for t in range(2):
            for hh, sel in ((2 * t, ev), (2 * t, od), (2 * t + 1, ev), (2 * t + 1, od)):
                qk_idx.append(DK * (4 * g + hh) + sel)
        qk_idx = np.concatenate(qk_idx)
        v_idx = 256 * g + np.arange(256)
        wq = Wq[qk_idx, :].T
        wk = Wk[qk_idx, :].T
        wv = Wv[v_idx, :].T
        wqkv = np.ascontiguousarray(
            np.concatenate([wq, wk, wv], axis=1)
        ).astype(NPBF16)
        woTl = np.ascontiguousarray(Wo[:, v_idx].T).astype(NPBF16)
        in_maps.append(
            dict(xT=xTb, wqkv=wqkv, woT=woTl, ropec=ropec, ropes=ropes,
                 trimask=trimask)
        )
    return in_maps


def _run(inputs, trace=False, tmpdir=None):
    if "nc" not in _CACHE:
        _CACHE["nc"] = _build_nc()
    nc = _CACHE["nc"]
    in_maps = _host_inputs(**inputs)
    kw = {}
    if trace:
        kw = dict(trace=True, tmpdir=tmpdir)
    res = run_bass_kernel_spmd(nc, in_maps, list(range(8)), **kw)
    out = np.zeros((2, S, D), np.float32)
    for core in range(8):
        out[core // 4] += res.results[core]["yp"].astype(np.float32)
    return out, res


def kernel(**inputs):
    out, _ = _run(inputs, trace=False)
    return out


# revision 44
# speedup vs baseline: 1.0349x; 1.0349x over previous
"""Multi-head self-attention (RoPE + causal) Trainium2 Bass kernel.

Problem: b=2, s=2048, d_model=1024, 16 heads x 64 dims, causal, RoPE.
Sharding over 8 NeuronCores: core c -> (batch bi = c//4, head group g = c%4
of 4 heads). Each core computes its 4 heads' attention from x[bi] and
produces a partial output projection (Wo column-block); the host sums the
4 partials per batch element.

v3: single interleaved pipeline tuned to keep the PE HAM-warm (2.4 GHz):
per s-quarter c: QK proj -> rope -> attention (qi=c), with V-projection and
out-projection tiles of the previous quarter emitted as PE fillers inside
the attention ladder (scores run one block ahead of AV so the PE never
waits on the ACT exp). Inputs arrive via consolidated multi-dim DMAs.
Causal diag masking is a bf16 triangular-mask multiply on DVE. Softmax
division: denominator row staged to partition 0 (custom-DVE reciprocal
mishandles nonzero partition bases), reciprocal, gpsimd partition
broadcast, multiply.

Per-core device layout (all matmul operands bf16, fp32 PSUM accumulate):
  x_all  [128, 4*8*512]   xT quarters: [quarter][kc][512 cols]
  w_qk   [128, 8*512]     per kc: [Q pair0 | Q pair1 | K pair0 | K pair1]
                          rows permuted per pair: [h even, h odd, h' even,
                          h' odd] (32 rows each) so RoPE is a half-swap +
                          elementwise mul/add
  QT/KT  [128, 2048]x2    projected+roped, tile t holds heads 2t, 2t+1
  scores ST[k,q] via PE (contraction = head dims, row-group packed pairs)
  softmax: exp on ACT (scale=1/8 folded in), causal mask = multiplicative
           bf16 upper-tri tile, denominator = ones column appended to V
           (row 64 of the AV PSUM accumulator)
  out    [2048, 1024] bf16 partial = O @ Wo_block
"""

import os
import sys
from contextlib import ExitStack

import numpy as np

for _p in ("/root/.axon_site", "/root/.axon_site/_ro/trn_rl_repo", "/opt/trn_rl_repo"):
    if os.path.isdir(_p) and _p not in sys.path:
        sys.path.append(_p)

import ml_dtypes  # noqa: E402
import concourse.bass as bass  # noqa: E402
import concourse.tile as tile  # noqa: E402
import concourse.mybir as mybir  # noqa: E402
from concourse import bacc  # noqa: E402
from concourse.bass import ts  # noqa: E402
from concourse.bass_utils import run_bass_kernel_spmd  # noqa: E402

BF16 = mybir.dt.bfloat16
F32 = mybir.dt.float32
NPBF16 = ml_dtypes.bfloat16

S = 2048
D = 1024
DK = 64
THETA = 10000.0

_CACHE = {}


def _build_nc(debug_out=False):
    nc = bacc.Bacc("TRN2", target_bir_lowering=False, debug=False, num_devices=8)
    xT = nc.dram_tensor("xT", [D, S], BF16, kind="ExternalInput").ap()
    wqkv = nc.dram_tensor("wqkv", [D, 768], BF16, kind="ExternalInput").ap()
    woT = nc.dram_tensor("woT", [256, D], BF16, kind="ExternalInput").ap()
    ropec = nc.dram_tensor("ropec", [128, 2048], BF16, kind="ExternalInput").ap()
    ropes = nc.dram_tensor("ropes", [128, 2048], BF16, kind="ExternalInput").ap()
    trimask = nc.dram_tensor("trimask", [128, 128], BF16, kind="ExternalInput").ap()
    yp = nc.dram_tensor("yp", [S, D], BF16, kind="ExternalOutput").ap()
    dbg = {}
    if debug_out:
        for nm in ("qf0", "kf0", "ot0", "ot1"):
            dbg[nm] = nc.dram_tensor(nm, [128, S], BF16, kind="ExternalOutput").ap()

    Exp = mybir.ActivationFunctionType.Exp

    with ExitStack() as ctx:
        tc = ctx.enter_context(tile.TileContext(nc))
        const = ctx.enter_context(tc.tile_pool(name="const", bufs=1))
        sb = ctx.enter_context(tc.tile_pool(name="sb", bufs=2))
        expp = ctx.enter_context(tc.tile_pool(name="expp", bufs=4))
        outp = ctx.enter_context(tc.tile_pool(name="outp", bufs=3))
        rp = ctx.enter_context(tc.tile_pool(name="rp", bufs=4))
        psA = ctx.enter_context(tc.tile_pool(name="psA", bufs=2, space="PSUM"))
        psB = ctx.enter_context(tc.tile_pool(name="psB", bufs=4, space="PSUM"))

        # ---- persistent SBUF ----
        x_all = const.tile([128, 8 * 2048], BF16, tag="x_all")
        w_all = const.tile([128, 8 * 768], BF16, tag="w_all")
        wo_all = const.tile([128, 2 * 1024], BF16, tag="wo_all")
        ropec_sb = const.tile([128, 2048], BF16, tag="ropec")
        ropes_sb = const.tile([128, 2048], BF16, tag="ropes")
        trimask_sb = const.tile([128, 128], BF16, tag="trimask")
        # V slots are 128 wide: [ones | 63 pad | 64 dims] so the AV PSUM
        # accumulator has the denominator on partition 0 (custom-DVE recip
        # requires base 0) and the dims at base 64 (aligned 64-partition read)
        v_sb = const.tile([128, 16 * 512], BF16, tag="v")
        qf = [const.tile([128, S], BF16, tag=f"qf{t}", name=f"qf{t}") for t in range(2)]
        kf = [const.tile([128, S], BF16, tag=f"kf{t}", name=f"kf{t}") for t in range(2)]
        ot = [const.tile([128, S], BF16, tag=f"ot{t}", name=f"ot{t}") for t in range(2)]

        def xq(kc, c):  # x columns for quarter c, contraction tile kc
            o = kc * 2048 + c * 512
            return x_all[:, o : o + 512]

        def xst(kc, st):  # x columns for s-tile st (128 wide)
            o = kc * 2048 + st * 128
            return x_all[:, o : o + 128]

        def wqk(kc, t, qk):  # Q (qk=0) / K (qk=1) weight tile for pair t
            o = kc * 768 + qk * 256 + t * 128
            return w_all[:, o : o + 128]

        def wv(kc):
            return w_all[:, kc * 768 + 512 : kc * 768 + 768]

        def wo(cc, nh):
            o = cc * 1024 + nh * 512
            return wo_all[:, o : o + 512]

        # ---- input DMAs: per-kc contiguous tiles (wide lines, full DMA
        # bandwidth), spread across the SP and ACT HW-DGE queues ----
        for kc in range(8):
            eng = nc.sync if kc % 2 == 0 else nc.scalar
            eng.dma_start(w_all[:, kc * 768 : (kc + 1) * 768], wqkv[ts(kc, 128), :])
        for kc in range(8):
            eng = nc.sync if kc % 2 == 0 else nc.scalar
            eng.dma_start(x_all[:, kc * 2048 : (kc + 1) * 2048], xT[ts(kc, 128), :])
        nc.sync.dma_start(ropec_sb[:], ropec[:])
        nc.scalar.dma_start(ropes_sb[:], ropes[:])
        nc.sync.dma_start(trimask_sb[:], trimask[:])
        for i in range(2):
            nc.scalar.dma_start(wo_all[:, ts(i, 1024)], woT[ts(i, 128), :])

        # v_sb slot headers: ones at col 0, zero pad at cols 1:32
        v_4d = v_sb[:].rearrange("p (b h x) -> p b h x", b=16, h=4)
        nc.gpsimd.memset(v_4d[:, :, :, 0:1], 1.0)
        nc.gpsimd.memset(v_4d[:, :, :, 1:64], 0.0)

        # deferred divide state from the previous (c, t) attention pass
        pend_div = []  # list of (qi, t, [oa0, oa1], [r0, r1])

        def emit_divides(upto):
            """Emit pbcast+mul for pending divides (recip already emitted)."""
            while len(pend_div) > upto:
                qi, t, oas, rs = pend_div.pop(0)
                for hh in range(2):
                    rb = rp.tile([64, 512], F32, tag="rb", name="rb")
                    nc.gpsimd.partition_broadcast(rb[:], rs[hh][0:1, :])
                    nc.vector.tensor_mul(
                        ot[t][64 * hh : 64 * hh + 64, ts(qi, 512)],
                        oas[hh][64:128, :],
                        rb[:],
                    )

        def emit_vproj(st):
            vp = psA.tile([128, 1024], F32, tag="psa", name="vp")
            for kc in range(8):
                nc.tensor.matmul(
                    vp[:, 0:256],
                    lhsT=xst(kc, st),
                    rhs=wv(kc),
                    start=(kc == 0),
                    stop=(kc == 7),
                )
            dst = v_sb[:, st * 512 : (st + 1) * 512].rearrange(
                "p (h x) -> p h x", h=4
            )[:, :, 64:128]
            vsrc = vp[:, 0:256].rearrange("p (h x) -> p h x", h=4)
            nc.vector.tensor_copy(dst, vsrc)

        def emit_outproj(st):
            pp = psA.tile([128, 1024], F32, tag="psa", name="pp")
            for nh in range(2):
                for cc in range(2):
                    nc.tensor.matmul(
                        pp[:, ts(nh, 512)],
                        lhsT=ot[cc][:, ts(st, 128)],
                        rhs=wo(cc, nh),
                        start=(cc == 0),
                        stop=(cc == 1),
                    )
            ob = outp.tile([128, 1024], BF16, tag="ob", name="ob")
            nc.scalar.copy(ob[:, 0:512], pp[:, 0:512])
            nc.vector.tensor_copy(ob[:, 512:1024], pp[:, 512:1024])
            nc.sync.dma_start(yp[ts(st, 128), :], ob[:])

        def qkproj_fillers(c, t):
            """Q/K projection + RoPE for quarter c, head pair t, as two PE
            filler chunks (Q half, then K half + the rope chain). The rope
            elementwise chain runs on DVE; the half-swap runs as SBUF->SBUF
            DMAs to keep it off the busy compute engines."""
            st8 = {}

            def f_q():
                sp = psA.tile([128, 1024], F32, tag="psa", name="sp")
                st8["sp"] = sp
                for kc in range(8):
                    nc.tensor.matmul(
                        sp[:, 0:512],
                        lhsT=wqk(kc, t, 0),
                        rhs=xq(kc, c),
                        start=(kc == 0),
                        stop=(kc == 7),
                    )

            def f_k():
                sp = st8["sp"]
                for kc in range(8):
                    nc.tensor.matmul(
                        sp[:, 512:1024],
                        lhsT=wqk(kc, t, 1),
                        rhs=xq(kc, c),
                        start=(kc == 0),
                        stop=(kc == 7),
                    )
                qb = sb.tile([128, 1024], BF16, tag="qb", name="qb")
                nc.vector.tensor_copy(qb[:], sp[:])
                wb = sb.tile([128, 1024], BF16, tag="wb", name="wb")
                nc.sync.dma_start(wb[0:32, :], qb[32:64, :])
                nc.sync.dma_start(wb[32:64, :], qb[0:32, :])
                nc.sync.dma_start(wb[64:96, :], qb[96:128, :])
                nc.sync.dma_start(wb[96:128, :], qb[64:96, :])
                t1 = sb.tile([128, 1024], BF16, tag="t1", name="t1")
                nc.vector.tensor_mul(t1[:, 0:512], qb[:, 0:512], ropec_sb[:, ts(c, 512)])
                nc.vector.tensor_mul(t1[:, 512:1024], qb[:, 512:1024], ropec_sb[:, ts(c, 512)])
                t2 = sb.tile([128, 1024], BF16, tag="t2", name="t2")
                nc.vector.tensor_mul(t2[:, 0:512], wb[:, 0:512], ropes_sb[:, ts(c, 512)])
                nc.vector.tensor_mul(t2[:, 512:1024], wb[:, 512:1024], ropes_sb[:, ts(c, 512)])
                nc.vector.tensor_add(qf[t][:, ts(c, 512)], t1[:, 0:512], t2[:, 0:512])
                nc.vector.tensor_add(kf[t][:, ts(c, 512)], t1[:, 512:1024], t2[:, 512:1024])

            return [f_q, f_k]

        # PE filler work queue, drained one item per attention block; the
        # next quarter's projections run as fillers inside this quarter's
        # ladder so the PE stays dense while ACT paces the exps. Leftovers
        # drain before the next ladder (whose scores depend on them).
        fillers = []

        def emit_filler():
            if fillers:
                fillers.pop(0)()

        # quarter-0 prologue
        for f in qkproj_fillers(0, 0) + qkproj_fillers(0, 1):
            f()
        for st in range(4):
            emit_vproj(st)

        for c in range(4):
            while fillers:
                emit_filler()
            if c < 3:
                fillers.extend(qkproj_fillers(c + 1, 0))
                fillers.extend(qkproj_fillers(c + 1, 1))
                fillers.extend(
                    [lambda st=st: emit_vproj(st) for st in range(4 * c + 4, 4 * c + 8)]
                )
            if c > 0:
                fillers.extend(
                    [lambda st=st: emit_outproj(st) for st in range(4 * (c - 1), 4 * c)]
                )

            # finish divides for the previous quarter
            emit_divides(0)

            # ---- attention for q-quarter qi=c, both head pairs merged into
            # one ladder (independent pairs interleave, so the PE never waits
            # on an ACT exp and there is no per-pair boundary bubble) ----
            qi = c
            nblk = 4 * qi + 4
            oab = [
                [psB.tile([128, 512], F32, tag="psb", name=f"oa{t}{_}") for _ in range(2)]
                for t in range(2)
            ]
            essb = [[None] * nblk, [None] * nblk]

            def emit_scores(t, j):
                dd = j - 4 * qi
                nn = 512 if dd < 0 else 512 - 128 * dd
                c0 = 512 - nn
                sp = psA.tile([128, 1024], F32, tag="psa", name="sc")
                for hh in range(2):
                    r0 = 64 * hh
                    nc.tensor.matmul(
                        sp[:, hh * 512 : hh * 512 + nn],
                        lhsT=kf[t][r0 : r0 + 64, ts(j, 128)],
                        rhs=qf[t][r0 : r0 + 64, qi * 512 + c0 : (qi + 1) * 512],
                        start=True,
                        stop=True,
                    )
                es = expp.tile([128, 1024], BF16, tag="es", name="es")
                essb[t][j] = es
                sp_v = sp[:].rearrange("p (b x) -> p b x", b=2)[:, :, 0:nn]
                es_v = es[:].rearrange("p (b x) -> p b x", b=2)[:, :, 0:nn]
                nc.scalar.activation(es_v, sp_v, Exp, scale=0.125)
                if dd >= 0:
                    for hh in range(2):
                        nc.vector.tensor_mul(
                            es[:, hh * 512 : hh * 512 + 128],
                            es[:, hh * 512 : hh * 512 + 128],
                            trimask_sb[:],
                        )

            def emit_av(t, j):
                dd = j - 4 * qi
                nn = 512 if dd < 0 else 512 - 128 * dd
                c0 = 512 - nn
                es = essb[t][j]
                for hh in range(2):
                    h = 2 * t + hh
                    off = j * 512 + h * 128
                    nc.tensor.matmul(
                        oab[t][hh][:, c0:512],
                        lhsT=v_sb[:, off : off + 128],
                        rhs=es[:, hh * 512 : hh * 512 + nn],
                        start=(j == 0),
                        stop=(j == nblk - 1),
                    )

            def emit_recips(t):
                # denominator reciprocals now (straight off the accumulator's
                # partition-0 row); pbcast+mul deferred so the PE can roll
                # into the next phase without waiting
                rs = []
                for hh in range(2):
                    r = rp.tile([1, 512], F32, tag="r", name="r")
                    nc.vector.reciprocal_approx_fast(r[:], oab[t][hh][0:1, 0:512])
                    rs.append(r)
                pend_div.append((qi, t, oab[t], rs))

            # split ladders per head pair, scores one block ahead of AV, with
            # a one-block lookahead across the pair boundary so the PE rolls
            # straight from pair 0's last AV into pair 1's first AV
            emit_scores(0, 0)
            for j in range(1, nblk):
                emit_scores(0, j)
                emit_filler()
                emit_av(0, j - 1)
            emit_scores(1, 0)
            emit_av(0, nblk - 1)
            emit_recips(0)
            for j in range(1, nblk):
                emit_scores(1, j)
                emit_filler()
                emit_av(1, j - 1)
            emit_av(1, nblk - 1)
            emit_recips(1)

        emit_divides(0)
        while fillers:
            emit_filler()
        for st in range(12, 16):
            emit_outproj(st)

        if debug_out:
            nc.sync.dma_start(dbg["qf0"][:], qf[0][:])
            nc.sync.dma_start(dbg["kf0"][:], kf[0][:])
            nc.sync.dma_start(dbg["ot0"][:], ot[0][:])
            nc.sync.dma_start(dbg["ot1"][:], ot[1][:])

    nc.compile()
    return nc


def _host_inputs(x, token_positions, Wq, Wk, Wv, Wo):
    x = np.asarray(x, dtype=np.float32)
    Wq = np.asarray(Wq, dtype=np.float32)
    Wk = np.asarray(Wk, dtype=np.float32)
    Wv = np.asarray(Wv, dtype=np.float32)
    Wo = np.asarray(Wo, dtype=np.float32)
    pos = np.asarray(token_positions).astype(np.float32)

    # rope tables, rows = [even(32) odd(32) even(32) odd(32)] freq index p%32
    f = np.arange(32, dtype=np.float32)
    inv = 1.0 / (THETA ** (2.0 * f / DK))
    ang = pos[:, None] * inv[None, :]  # [S, 32]
    cosT = np.cos(ang).T.astype(np.float32)  # [32, S]
    sinT = np.sin(ang).T.astype(np.float32)
    crow = np.tile(cosT, (4, 1))
    srow = np.concatenate([-sinT, sinT, -sinT, sinT], axis=0)

    ropec = np.ascontiguousarray(crow).astype(NPBF16)
    ropes = np.ascontiguousarray(srow).astype(NPBF16)
    trimask = np.triu(np.ones((128, 128), dtype=np.float32)).astype(NPBF16)

    ev = np.arange(0, DK, 2)
    od = np.arange(1, DK, 2)
    in_maps = []
    for core in range(8):
        bi, g = core // 4, core % 4
        xTb = np.ascontiguousarray(x[bi].T).astype(NPBF16)
        qk_idx = []
        for t in range(2):
            for hh, sel in ((2 * t, ev), (2 * t, od), (2 * t + 1, ev), (2 * t + 1, od)):
                qk_idx.append(DK * (4 * g + hh) + sel)
        qk_idx = np.concatenate(qk_idx)
        v_idx = 256 * g + np.arange(256)
        wq = Wq[qk_idx, :].T
        wk = Wk[qk_idx, :].T
        wv = Wv[v_idx, :].T
        wqkv = np.ascontiguousarray(
            np.concatenate([wq, wk, wv], axis=1)
        ).astype(NPBF16)
        woTl = np.ascontiguousarray(Wo[:, v_idx].T).astype(NPBF16)
        in_maps.append(
            dict(xT=xTb, wqkv=wqkv, woT=woTl, ropec=ropec, ropes=ropes,
                 trimask=trimask)
        )
    return in_maps


def _run(inputs, trace=False, tmpdir=None):
    if "nc" not in _CACHE:
        _CACHE["nc"] = _build_nc()
    nc = _CACHE["nc"]
    in_maps = _host_inputs(**inputs)
    kw = {}
    if trace:
        kw = dict(trace=True, tmpdir=tmpdir)
    res = run_bass_kernel_spmd(nc, in_maps, list(range(8)), **kw)
    out = np.zeros((2, S, D), np.float32)
    for core in range(8):
        out[core // 4] += res.results[core]["yp"].astype(np.float32)
    return out, res


def kernel(**inputs):
    out, _ = _run(inputs, trace=False)
    return out


# revision 45
# speedup vs baseline: 1.0639x; 1.0280x over previous
"""Multi-head self-attention (RoPE + causal) Trainium2 Bass kernel.

Problem: b=2, s=2048, d_model=1024, 16 heads x 64 dims, causal, RoPE.
Sharding over 8 NeuronCores: core c -> (batch bi = c//4, head group g = c%4
of 4 heads). Each core computes its 4 heads' attention from x[bi] and
produces a partial output projection (Wo column-block); the host sums the
4 partials per batch element.

v3: single interleaved pipeline tuned to keep the PE HAM-warm (2.4 GHz):
per s-quarter c: QK proj -> rope -> attention (qi=c), with V-projection and
out-projection tiles of the previous quarter emitted as PE fillers inside
the attention ladder (scores run one block ahead of AV so the PE never
waits on the ACT exp). Inputs arrive via consolidated multi-dim DMAs.
Causal diag masking is a bf16 triangular-mask multiply on DVE. Softmax
division: denominator row staged to partition 0 (custom-DVE reciprocal
mishandles nonzero partition bases), reciprocal, gpsimd partition
broadcast, multiply.

Per-core device layout (all matmul operands bf16, fp32 PSUM accumulate):
  x_all  [128, 4*8*512]   xT quarters: [quarter][kc][512 cols]
  w_qk   [128, 8*512]     per kc: [Q pair0 | Q pair1 | K pair0 | K pair1]
                          rows permuted per pair: [h even, h odd, h' even,
                          h' odd] (32 rows each) so RoPE is a half-swap +
                          elementwise mul/add
  QT/KT  [128, 2048]x2    projected+roped, tile t holds heads 2t, 2t+1
  scores ST[k,q] via PE (contraction = head dims, row-group packed pairs)
  softmax: exp on ACT (scale=1/8 folded in), causal mask = multiplicative
           bf16 upper-tri tile, denominator = ones column appended to V
           (row 64 of the AV PSUM accumulator)
  out    [2048, 1024] bf16 partial = O @ Wo_block
"""

import os
import sys
from contextlib import ExitStack

import numpy as np

for _p in ("/root/.axon_site", "/root/.axon_site/_ro/trn_rl_repo", "/opt/trn_rl_repo"):
    if os.path.isdir(_p) and _p not in sys.path:
        sys.path.append(_p)

import ml_dtypes  # noqa: E402
import concourse.bass as bass  # noqa: E402
import concourse.tile as tile  # noqa: E402
import concourse.mybir as mybir  # noqa: E402
from concourse import bacc  # noqa: E402
from concourse.bass import ts  # noqa: E402
from concourse.bass_utils import run_bass_kernel_spmd  # noqa: E402

BF16 = mybir.dt.bfloat16
F32 = mybir.dt.float32
NPBF16 = ml_dtypes.bfloat16

S = 2048
D = 1024
DK = 64
THETA = 10000.0

_CACHE = {}


def _build_nc(debug_out=False):
    nc = bacc.Bacc("TRN2", target_bir_lowering=False, debug=False, num_devices=8)
    xT = nc.dram_tensor("xT", [D, S], BF16, kind="ExternalInput").ap()
    wqkv = nc.dram_tensor("wqkv", [D, 768], BF16, kind="ExternalInput").ap()
    woT = nc.dram_tensor("woT", [256, D], BF16, kind="ExternalInput").ap()
    ropec = nc.dram_tensor("ropec", [128, 2048], BF16, kind="ExternalInput").ap()
    ropes = nc.dram_tensor("ropes", [128, 2048], BF16, kind="ExternalInput").ap()
    trimask = nc.dram_tensor("trimask", [128, 128], BF16, kind="ExternalInput").ap()
    yp = nc.dram_tensor("yp", [S, D], BF16, kind="ExternalOutput").ap()
    dbg = {}
    if debug_out:
        for nm in ("qf0", "kf0", "ot0", "ot1"):
            dbg[nm] = nc.dram_tensor(nm, [128, S], BF16, kind="ExternalOutput").ap()

    Exp = mybir.ActivationFunctionType.Exp

    with ExitStack() as ctx:
        tc = ctx.enter_context(tile.TileContext(nc))
        const = ctx.enter_context(tc.tile_pool(name="const", bufs=1))
        sb = ctx.enter_context(tc.tile_pool(name="sb", bufs=2))
        expp = ctx.enter_context(tc.tile_pool(name="expp", bufs=4))
        outp = ctx.enter_context(tc.tile_pool(name="outp", bufs=3))
        rp = ctx.enter_context(tc.tile_pool(name="rp", bufs=4))
        psA = ctx.enter_context(tc.tile_pool(name="psA", bufs=2, space="PSUM"))
        psB = ctx.enter_context(tc.tile_pool(name="psB", bufs=4, space="PSUM"))

        # ---- persistent SBUF ----
        x_all = const.tile([128, 8 * 2048], BF16, tag="x_all")
        w_all = const.tile([128, 8 * 768], BF16, tag="w_all")
        wo_all = const.tile([128, 2 * 1024], BF16, tag="wo_all")
        ropec_sb = const.tile([128, 2048], BF16, tag="ropec")
        ropes_sb = const.tile([128, 2048], BF16, tag="ropes")
        trimask_sb = const.tile([128, 128], BF16, tag="trimask")
        # V slots are 128 wide: [ones | 63 pad | 64 dims] so the AV PSUM
        # accumulator has the denominator on partition 0 (custom-DVE recip
        # requires base 0) and the dims at base 64 (aligned 64-partition read)
        v_sb = const.tile([128, 16 * 512], BF16, tag="v")
        qf = [const.tile([128, S], BF16, tag=f"qf{t}", name=f"qf{t}") for t in range(2)]
        kf = [const.tile([128, S], BF16, tag=f"kf{t}", name=f"kf{t}") for t in range(2)]
        ot = [const.tile([128, S], BF16, tag=f"ot{t}", name=f"ot{t}") for t in range(2)]

        def xq(kc, c):  # x columns for quarter c, contraction tile kc
            o = kc * 2048 + c * 512
            return x_all[:, o : o + 512]

        def xst(kc, st):  # x columns for s-tile st (128 wide)
            o = kc * 2048 + st * 128
            return x_all[:, o : o + 128]

        def wqk(kc, t, qk):  # Q (qk=0) / K (qk=1) weight tile for pair t
            o = kc * 768 + qk * 256 + t * 128
            return w_all[:, o : o + 128]

        def wv(kc):
            return w_all[:, kc * 768 + 512 : kc * 768 + 768]

        def wo(cc, nh):
            o = cc * 1024 + nh * 512
            return wo_all[:, o : o + 512]

        # ---- input DMAs: per-kc contiguous slices (wide lines, full DMA
        # bandwidth), quarter-0 x first, spread across the two HW-DGE
        # queues; the bulk x tail goes on the ACT queue only so the SP
        # queue stays clear for the latency-critical rope swap DMAs ----
        for kc in range(8):
            eng = nc.sync if kc % 2 == 0 else nc.scalar
            eng.dma_start(w_all[:, kc * 768 : (kc + 1) * 768], wqkv[ts(kc, 128), :])
        for kc in range(8):
            eng = nc.sync if kc % 2 == 0 else nc.scalar
            eng.dma_start(x_all[:, kc * 2048 : kc * 2048 + 512], xT[ts(kc, 128), 0:512])
        nc.sync.dma_start(ropec_sb[:], ropec[:])
        nc.scalar.dma_start(ropes_sb[:], ropes[:])
        nc.sync.dma_start(trimask_sb[:], trimask[:])
        for i in range(2):
            nc.scalar.dma_start(wo_all[:, ts(i, 1024)], woT[ts(i, 128), :])
        for kc in range(8):
            nc.scalar.dma_start(
                x_all[:, kc * 2048 + 512 : (kc + 1) * 2048],
                xT[ts(kc, 128), 512:2048],
            )

        # v_sb slot headers: ones at col 0, zero pad at cols 1:32
        v_4d = v_sb[:].rearrange("p (b h x) -> p b h x", b=16, h=4)
        nc.gpsimd.memset(v_4d[:, :, :, 0:1], 1.0)
        nc.gpsimd.memset(v_4d[:, :, :, 1:64], 0.0)

        # deferred divide state from the previous (c, t) attention pass
        pend_div = []  # list of (qi, t, [oa0, oa1], [r0, r1])

        def emit_divides(upto):
            """Emit pbcast+mul for pending divides (recip already emitted)."""
            while len(pend_div) > upto:
                qi, t, oas, rs = pend_div.pop(0)
                for hh in range(2):
                    rb = rp.tile([64, 512], F32, tag="rb", name="rb")
                    nc.gpsimd.partition_broadcast(rb[:], rs[hh][0:1, :])
                    nc.vector.tensor_mul(
                        ot[t][64 * hh : 64 * hh + 64, ts(qi, 512)],
                        oas[hh][64:128, :],
                        rb[:],
                    )

        def emit_vproj(st):
            vp = psA.tile([128, 1024], F32, tag="psa", name="vp")
            for kc in range(8):
                nc.tensor.matmul(
                    vp[:, 0:256],
                    lhsT=xst(kc, st),
                    rhs=wv(kc),
                    start=(kc == 0),
                    stop=(kc == 7),
                )
            dst = v_sb[:, st * 512 : (st + 1) * 512].rearrange(
                "p (h x) -> p h x", h=4
            )[:, :, 64:128]
            vsrc = vp[:, 0:256].rearrange("p (h x) -> p h x", h=4)
            nc.vector.tensor_copy(dst, vsrc)

        def emit_outproj(st):
            pp = psA.tile([128, 1024], F32, tag="psa", name="pp")
            for nh in range(2):
                for cc in range(2):
                    nc.tensor.matmul(
                        pp[:, ts(nh, 512)],
                        lhsT=ot[cc][:, ts(st, 128)],
                        rhs=wo(cc, nh),
                        start=(cc == 0),
                        stop=(cc == 1),
                    )
            ob = outp.tile([128, 1024], BF16, tag="ob", name="ob")
            nc.vector.tensor_copy(ob[:], pp[:])
            nc.sync.dma_start(yp[ts(st, 128), :], ob[:])

        def qkproj_fillers(c, t):
            """Q/K projection + RoPE for quarter c, head pair t, as two PE
            filler chunks (Q half, then K half + the rope chain). The rope
            elementwise chain runs on DVE; the half-swap runs as SBUF->SBUF
            DMAs to keep it off the busy compute engines."""
            st8 = {}

            def f_q():
                sp = psA.tile([128, 1024], F32, tag="psa", name="sp")
                st8["sp"] = sp
                for kc in range(8):
                    nc.tensor.matmul(
                        sp[:, 0:512],
                        lhsT=wqk(kc, t, 0),
                        rhs=xq(kc, c),
                        start=(kc == 0),
                        stop=(kc == 7),
                    )

            def f_k():
                sp = st8["sp"]
                for kc in range(8):
                    nc.tensor.matmul(
                        sp[:, 512:1024],
                        lhsT=wqk(kc, t, 1),
                        rhs=xq(kc, c),
                        start=(kc == 0),
                        stop=(kc == 7),
                    )
                qb = sb.tile([128, 1024], BF16, tag="qb", name="qb")
                nc.vector.tensor_copy(qb[:], sp[:])
                wb = sb.tile([128, 1024], BF16, tag="wb", name="wb")
                nc.sync.dma_start(wb[0:32, :], qb[32:64, :])
                nc.sync.dma_start(wb[32:64, :], qb[0:32, :])
                nc.sync.dma_start(wb[64:96, :], qb[96:128, :])
                nc.sync.dma_start(wb[96:128, :], qb[64:96, :])
                t1 = sb.tile([128, 1024], BF16, tag="t1", name="t1")
                nc.vector.tensor_mul(t1[:, 0:512], qb[:, 0:512], ropec_sb[:, ts(c, 512)])
                nc.vector.tensor_mul(t1[:, 512:1024], qb[:, 512:1024], ropec_sb[:, ts(c, 512)])
                t2 = sb.tile([128, 1024], BF16, tag="t2", name="t2")
                nc.vector.tensor_mul(t2[:, 0:512], wb[:, 0:512], ropes_sb[:, ts(c, 512)])
                nc.vector.tensor_mul(t2[:, 512:1024], wb[:, 512:1024], ropes_sb[:, ts(c, 512)])
                nc.vector.tensor_add(qf[t][:, ts(c, 512)], t1[:, 0:512], t2[:, 0:512])
                nc.vector.tensor_add(kf[t][:, ts(c, 512)], t1[:, 512:1024], t2[:, 512:1024])

            return [f_q, f_k]

        # PE filler work queue, drained one item per attention block; the
        # next quarter's projections run as fillers inside this quarter's
        # ladder so the PE stays dense while ACT paces the exps. Leftovers
        # drain before the next ladder (whose scores depend on them).
        fillers = []

        def emit_filler():
            if fillers:
                fillers.pop(0)()

        # quarter-0 prologue
        for f in qkproj_fillers(0, 0) + qkproj_fillers(0, 1):
            f()
        for st in range(4):
            emit_vproj(st)

        for c in range(4):
            while fillers:
                emit_filler()
            if c < 3:
                fillers.extend(qkproj_fillers(c + 1, 0))
                fillers.extend(qkproj_fillers(c + 1, 1))
                fillers.extend(
                    [lambda st=st: emit_vproj(st) for st in range(4 * c + 4, 4 * c + 8)]
                )
            if c > 0:
                fillers.extend(
                    [lambda st=st: emit_outproj(st) for st in range(4 * (c - 1), 4 * c)]
                )

            # finish divides for the previous quarter
            emit_divides(0)

            # ---- attention for q-quarter qi=c, both head pairs merged into
            # one ladder (independent pairs interleave, so the PE never waits
            # on an ACT exp and there is no per-pair boundary bubble) ----
            qi = c
            nblk = 4 * qi + 4
            oab = [
                [psB.tile([128, 512], F32, tag="psb", name=f"oa{t}{_}") for _ in range(2)]
                for t in range(2)
            ]
            essb = [[None] * nblk, [None] * nblk]

            def emit_scores(t, j):
                dd = j - 4 * qi
                nn = 512 if dd < 0 else 512 - 128 * dd
                c0 = 512 - nn
                sp = psA.tile([128, 1024], F32, tag="psa", name="sc")
                for hh in range(2):
                    r0 = 64 * hh
                    nc.tensor.matmul(
                        sp[:, hh * 512 : hh * 512 + nn],
                        lhsT=kf[t][r0 : r0 + 64, ts(j, 128)],
                        rhs=qf[t][r0 : r0 + 64, qi * 512 + c0 : (qi + 1) * 512],
                        start=True,
                        stop=True,
                    )
                es = expp.tile([128, 1024], BF16, tag="es", name="es")
                essb[t][j] = es
                sp_v = sp[:].rearrange("p (b x) -> p b x", b=2)[:, :, 0:nn]
                es_v = es[:].rearrange("p (b x) -> p b x", b=2)[:, :, 0:nn]
                nc.scalar.activation(es_v, sp_v, Exp, scale=0.125)
                if dd >= 0:
                    for hh in range(2):
                        nc.vector.tensor_mul(
                            es[:, hh * 512 : hh * 512 + 128],
                            es[:, hh * 512 : hh * 512 + 128],
                            trimask_sb[:],
                        )

            def emit_av(t, j):
                dd = j - 4 * qi
                nn = 512 if dd < 0 else 512 - 128 * dd
                c0 = 512 - nn
                es = essb[t][j]
                for hh in range(2):
                    h = 2 * t + hh
                    off = j * 512 + h * 128
                    nc.tensor.matmul(
                        oab[t][hh][:, c0:512],
                        lhsT=v_sb[:, off : off + 128],
                        rhs=es[:, hh * 512 : hh * 512 + nn],
                        start=(j == 0),
                        stop=(j == nblk - 1),
                    )

            def emit_recips(t):
                # denominator reciprocals now (straight off the accumulator's
                # partition-0 row); pbcast+mul deferred so the PE can roll
                # into the next phase without waiting
                rs = []
                for hh in range(2):
                    r = rp.tile([1, 512], F32, tag="r", name="r")
                    nc.vector.reciprocal_approx_fast(r[:], oab[t][hh][0:1, 0:512])
                    rs.append(r)
                pend_div.append((qi, t, oab[t], rs))

            # split ladders per head pair, scores one block ahead of AV, with
            # a one-block lookahead across the pair boundary so the PE rolls
            # straight from pair 0's last AV into pair 1's first AV
            emit_scores(0, 0)
            for j in range(1, nblk):
                emit_scores(0, j)
                emit_filler()
                emit_av(0, j - 1)
            emit_scores(1, 0)
            emit_av(0, nblk - 1)
            emit_recips(0)
            for j in range(1, nblk):
                emit_scores(1, j)
                emit_filler()
                emit_av(1, j - 1)
            emit_av(1, nblk - 1)
            emit_recips(1)

        emit_divides(0)
        while fillers:
            emit_filler()
        for st in range(12, 16):
            emit_outproj(st)

        if debug_out:
            nc.sync.dma_start(dbg["qf0"][:], qf[0][:])
            nc.sync.dma_start(dbg["kf0"][:], kf[0][:])
            nc.sync.dma_start(dbg["ot0"][:], ot[0][:])
            nc.sync.dma_start(dbg["ot1"][:], ot[1][:])

    nc.compile()
    return nc


def _host_inputs(x, token_positions, Wq, Wk, Wv, Wo):
    x = np.asarray(x, dtype=np.float32)
    Wq = np.asarray(Wq, dtype=np.float32)
    Wk = np.asarray(Wk, dtype=np.float32)
    Wv = np.asarray(Wv, dtype=np.float32)
    Wo = np.asarray(Wo, dtype=np.float32)
    pos = np.asarray(token_positions).astype(np.float32)

    # rope tables, rows = [even(32) odd(32) even(32) odd(32)] freq index p%32
    f = np.arange(32, dtype=np.float32)
    inv = 1.0 / (THETA ** (2.0 * f / DK))
    ang = pos[:, None] * inv[None, :]  # [S, 32]
    cosT = np.cos(ang).T.astype(np.float32)  # [32, S]
    sinT = np.sin(ang).T.astype(np.float32)
    crow = np.tile(cosT, (4, 1))
    srow = np.concatenate([-sinT, sinT, -sinT, sinT], axis=0)

    ropec = np.ascontiguousarray(crow).astype(NPBF16)
    ropes = np.ascontiguousarray(srow).astype(NPBF16)
    trimask = np.triu(np.ones((128, 128), dtype=np.float32)).astype(NPBF16)

    ev = np.arange(0, DK, 2)
    od = np.arange(1, DK, 2)
    in_maps = []
    for core in range(8):
        bi, g = core // 4, core % 4
        xTb = np.ascontiguousarray(x[bi].T).astype(NPBF16)
        qk_idx = []
        for t in range(2):
            for hh, sel in ((2 * t, ev), (2 * t, od), (2 * t + 1, ev), (2 * t + 1, od)):
                qk_idx.append(DK * (4 * g + hh) + sel)
        qk_idx = np.concatenate(qk_idx)
        v_idx = 256 * g + np.arange(256)
        wq = Wq[qk_idx, :].T
        wk = Wk[qk_idx, :].T
        wv = Wv[v_idx, :].T
        wqkv = np.ascontiguousarray(
            np.concatenate([wq, wk, wv], axis=1)
        ).astype(NPBF16)
        woTl = np.ascontiguousarray(Wo[:, v_idx].T).astype(NPBF16)
        in_maps.append(
            dict(xT=xTb, wqkv=wqkv, woT=woTl, ropec=ropec, ropes=ropes,
                 trimask=trimask)
        )
    return in_maps


def _run(inputs, trace=False, tmpdir=None):
    if "nc" not in _CACHE:
        _CACHE["nc"] = _build_nc()
    nc = _CACHE["nc"]
    in_maps = _host_inputs(**inputs)
    kw = {}
    if trace:
        kw = dict(trace=True, tmpdir=tmpdir)
    res = run_bass_kernel_spmd(nc, in_maps, list(range(8)), **kw)
    out = np.zeros((2, S, D), np.float32)
    for core in range(8):
        out[core // 4] += res.results[core]["yp"].astype(np.float32)
    return out, res


def kernel(**inputs):
    out, _ = _run(inputs, trace=False)
    return out
